# revision 34
# baseline (speedup 1.0000x reference)
"""DiffusionGraphConv Trainium2 kernel (bf16 v2).

Math (per batch b, support s, A = supports[s]):
  x0 = concat(inputs, state)                      # [N, F=128]
  reference out = sum_k x_k @ W_k  (+bias), k in {x0, x1_s0, x2_s0, x1_s1, x2_s1}
  with x1 = A x0, x2 = 2 A A x0 - x0, W_k = weight[f*5+k, :].

Restructured to avoid any on-chip transposes:
  out = x0 @ What + bias + sum_s A_s @ (x0 @ W1_s + A_s @ (x0 @ (2*W2_s)))
  with What = W_0 - W_2 - W_4 (host-prepped, as are the 2x scalings).

All matmul operands are bf16 (f32 PSUM accumulation): same 1 cycle/row
streaming as fp32r but half the DMA bytes (input head shrinks ~2x) and
FWL-accelerated LDWEIGHTS that hides fully under the streams.  Measured
end-to-end error vs the f32 reference is ~3.5e-3 on the max|err|/max|ref|
metric (gate 2e-2).

Layouts (per core, batch-sharded BL=8):
  x0tN  DRAM [b=8, F=128, m=1024] bf16  (host-staged transpose; lhsT tiles)
  atN   DRAM [s=2, m=1024, n=1024] bf16 (host-staged A^T; lhsT tiles)
  wcN   DRAM [F, 5, O] bf16 = [What, W1, 2*W2, W3, 2*W4]
  Sa step (b, mi): one matmul x0T-tile @ wc -> psum [128, 640]:
    [:,0:128] -> fins[mi] (f32 init), [:,128:640] -> st[:, mi, b, :] (bf16)
    st cols: [w1p_s0 | u_s0 | w1p_s1 | u_s1]
  v bank (s, ni, h):  v_s[ni, h] = A_s @ u_s + w1p_s   (psum f32 -> bf16)
  fin bank (s, ni, h): fin[ni, h] += A_s @ v_s; final s: DMA out [n, b, o]
"""

import sys as _sys
import types as _types

try:
    import antenv.axon_hooks  # noqa: F401
except Exception:
    try:
        import antenv as _antenv

        _m = _types.ModuleType("antenv.axon_hooks")
        _m._hook = None
        _m.set_axon_ntff_profile_hook = lambda h: setattr(_m, "_hook", h)
        _m.get_axon_ntff_profile_hook = lambda: _m._hook
        _sys.modules["antenv.axon_hooks"] = _m
        _antenv.axon_hooks = _m
    except Exception:
        pass

import ml_dtypes
import numpy as np

import concourse.mybir as mybir
import concourse.tile as tile
from concourse import bacc
from concourse.bass_utils import run_bass_kernel_spmd

NCORES = 8
B = 64
BL = B // NCORES  # 8 batches per core
N = 1024
F = 128
O = 128
NCH = N // 128  # 8 node chunks

BF16 = mybir.dt.bfloat16
F32 = mybir.dt.float32
NPBF16 = ml_dtypes.bfloat16

_CACHE = {}


def _build():
    if "nc" in _CACHE:
        return _CACHE["nc"]

    nc = bacc.Bacc(trn_type="TRN2", num_devices=NCORES, debug=False)

    x0t_d = nc.dram_tensor("x0t", [BL, F, N], BF16, kind="ExternalInput")
    at_d = nc.dram_tensor("at", [2, N, N], BF16, kind="ExternalInput")
    w_d = nc.dram_tensor("w", [F, 5, O], BF16, kind="ExternalInput")
    out_d = nc.dram_tensor("out", [N, BL, O], F32, kind="ExternalOutput")

    with tile.TileContext(nc) as tc:
        with (
            tc.tile_pool(name="big", bufs=1) as big,
            tc.tile_pool(name="small", bufs=1) as small,
            tc.tile_pool(name="ps_pool", bufs=1, space="PSUM") as ps_pool,
        ):
            # ---- persistent tiles ----
            wc = small.tile([F, 5, O], BF16)
            # x0t split into 4 tiles so early batches unblock Sa while the
            # rest still streams in (one DMA trigger per tile; triggers
            # serialize at ~0.6us each on an engine queue)
            x0t_g = [
                big.tile([F, 512], BF16, name="x0t_b0a"),
                big.tile([F, 512], BF16, name="x0t_b0b"),
                big.tile([F, 512], BF16, name="x0t_b1a"),
                big.tile([F, 512], BF16, name="x0t_b1b"),
                big.tile([F, 1, N], BF16, name="x0t_b2"),
                big.tile([F, 1, N], BF16, name="x0t_b3"),
                big.tile([F, 4, N], BF16, name="x0t_b47"),
            ]

            def x0chunk(b, mi):
                """[F, 128] lhsT tile for batch b, node chunk mi."""
                if b < 2:
                    g = 2 * b + (mi // 4)
                    return x0t_g[g][:, (mi % 4) * 128 : (mi % 4 + 1) * 128]
                if b < 4:
                    return x0t_g[2 + b][:, 0, mi * 128 : (mi + 1) * 128]
                return x0t_g[6][:, b - 4, mi * 128 : (mi + 1) * 128]

            at_t0 = big.tile([128, NCH, N], BF16, name="at_t0")
            at_t1 = big.tile([128, NCH, N], BF16, name="at_t1")
            # staging: [w1p_s0 | u_s0 | w1p_s1 | u_s1] per (mi, b)
            st = big.tile([128, NCH, BL, 512], BF16, name="st")
            v0 = big.tile([128, NCH, N], BF16, name="v0")
            v1 = big.tile([128, NCH, N], BF16, name="v1")
            fins = [big.tile([128, N], F32, name=f"fin{ni}") for ni in range(NCH)]

            # ---- PE warm-up: dummy matmuls during the DMA head so HAM
            # un-throttles (1.2 -> 2.4 GHz) before real work starts
            dummy = small.tile([128, 256], BF16)
            dsink = small.tile([128, 1], F32)
            nc.gpsimd.memset(dummy[:], 0.0)
            for _ in range(10):
                pw = ps_pool.tile([128, 256], F32, name="ps_w", tag="ps", bufs=8)
                nc.tensor.matmul(
                    pw[:], dummy[:, 0:128], dummy[:], start=True, stop=True
                )
            nc.vector.tensor_copy(dsink[:], pw[:, 0:1])

            # ---- input DMAs: 7 triggers, all on ONE queue so the data
            # streams in strict priority order at full aggregate bandwidth
            # (a second active queue steals DMA-engine bandwidth from the
            # Sa-critical x0t transfers).
            nc.sync.dma_start(wc[:], w_d[:])
            x0r = x0t_d.rearrange("b f n -> f b n")
            for g in range(4):
                b, hh = g // 2, g % 2
                nc.sync.dma_start(
                    x0t_g[g][:], x0r[:, b, hh * 512 : (hh + 1) * 512]
                )
            nc.sync.dma_start(x0t_g[4][:], x0r[:, 2:3, :])
            nc.sync.dma_start(x0t_g[5][:], x0r[:, 3:4, :])
            nc.sync.dma_start(x0t_g[6][:], x0r[:, 4:8, :])
            nc.sync.dma_start(
                at_t0[:], at_d[0].rearrange("(mi p) n -> p mi n", p=128)
            )
            nc.sync.dma_start(
                at_t1[:], at_d[1].rearrange("(mi p) n -> p mi n", p=128)
            )

            # ---- Sa step (b, mi): x0T tile stationary, wc moving; two
            # 256-free matmuls fill one single-bank psum with the four
            # staged products [W1 | 2W2 | W3 | 2W4], then ONE contiguous
            # 512-wide f32->bf16 cast lands them in st.  (The x0@What term
            # is folded into the fin0 accumulation groups instead.)
            def sa_step(b, mi):
                cnt = b * NCH + mi
                ps = ps_pool.tile([128, 512], F32, name="ps_sa", tag="ps", bufs=8)
                x0_tile = x0chunk(b, mi)
                nc.tensor.matmul(
                    ps[:, 0:256], x0_tile, wc[:, 1:3, :], start=True, stop=True
                )
                nc.tensor.matmul(
                    ps[:, 256:512], x0_tile, wc[:, 3:5, :], start=True, stop=True
                )
                if cnt % 2 == 0:
                    nc.scalar.copy(st[:, mi, b, :], ps[:])
                else:
                    nc.vector.tensor_copy(st[:, mi, b, :], ps[:])

            # ---- v bank (s, ni, h): v_s[ni, h] = A_s @ u_s + w1p_s
            def v_bank(s, at_t, v, ni, h):
                pv = ps_pool.tile([128, 512], F32, name="ps_v", tag="ps", bufs=8)
                for mi in range(NCH):
                    nc.tensor.matmul(
                        pv[:],
                        at_t[:, mi, ni * 128 : (ni + 1) * 128],
                        st[:, mi, 4 * h : 4 * h + 4, 256 * s + 128 : 256 * s + 256],
                        start=(mi == 0),
                        stop=(mi == NCH - 1),
                    )
                nc.vector.tensor_add(
                    v[:, ni, h * 512 : (h + 1) * 512],
                    pv[:],
                    st[:, ni, 4 * h : 4 * h + 4, 256 * s : 256 * s + 128],
                )

            # ---- fin bank (s, ni, h): fin[ni, h] += A_s @ v_s.
            # s=0 additionally accumulates the x0@What term (one 128-free
            # matmul per batch in the group) and writes fins fresh from the
            # psum; s=1 does fins += psum and DMAs the result out.  The very
            # last bank splits its add+DMA into 4 batch slices so the
            # exposed tail after the final matmul stays short.
            def fin_bank(s, at_t, v, ni, h):
                pf = ps_pool.tile([128, 512], F32, name="ps_f", tag="ps", bufs=8)
                for mi in range(NCH):
                    nc.tensor.matmul(
                        pf[:],
                        at_t[:, mi, ni * 128 : (ni + 1) * 128],
                        v[:, mi, h * 512 : (h + 1) * 512],
                        start=(mi == 0),
                        stop=(s == 1 and mi == NCH - 1),
                    )
                if s == 0:
                    for k in range(4):
                        b = 4 * h + k
                        nc.tensor.matmul(
                            pf[:, k * 128 : (k + 1) * 128],
                            x0chunk(b, ni),
                            wc[:, 0, :],
                            start=False,
                            stop=(k == 3),
                        )
                fslc = fins[ni][:, h * 512 : (h + 1) * 512]
                last = s == 1 and ni == NCH - 1
                if s == 0:
                    nc.vector.tensor_copy(fslc, pf[:])
                elif not last:
                    nc.vector.tensor_add(fslc, fslc, pf[:])
                    nc.sync.dma_start(
                        out_d[ni * 128 : (ni + 1) * 128, 4 * h : 4 * h + 4, :],
                        fslc,
                    )
                else:
                    # tail: one add, then the DMA split by node quarter
                    # (keeps 2KB-contiguous rows) fanned over idle engine
                    # queues so the triggers fire in parallel
                    nc.vector.tensor_add(fslc, fslc, pf[:])
                    trig = [nc.scalar, nc.gpsimd, nc.sync, nc.scalar]
                    for q in range(4):
                        p0 = q * 32
                        trig[q].dma_start(
                            out_d[
                                ni * 128 + p0 : ni * 128 + p0 + 32,
                                4 * h : 4 * h + 4,
                                :,
                            ],
                            fins[ni][p0 : p0 + 32, h * 512 : (h + 1) * 512],
                        )

            # ---- schedule ----
            # Sa b0-6 alone (runs during the x0t/at0 DMA), then v0 h=0
            # banks interleaved with Sa b7 (1 step per bank); delaying
            # v0 start this way lets the at0 DMA finish first.
            for b in range(7):
                for mi in range(NCH):
                    sa_step(b, mi)
            for ni in range(NCH):
                v_bank(0, at_t0, v0, ni, 0)
                sa_step(7, ni)
            for ni in range(NCH):
                v_bank(0, at_t0, v0, ni, 1)

            for ni in range(NCH):
                for h in range(2):
                    fin_bank(0, at_t0, v0, ni, h)

            for h in range(2):
                for ni in range(NCH):
                    v_bank(1, at_t1, v1, ni, h)
            for ni in range(NCH):
                for h in range(2):
                    fin_bank(1, at_t1, v1, ni, h)

    nc.compile()
    _CACHE["nc"] = nc
    return nc


def kernel(supports, inputs, state, weight, biases, output_size, _trace=False):
    supports = np.asarray(supports, dtype=np.float32)
    inputs = np.asarray(inputs, dtype=np.float32)
    state = np.asarray(state, dtype=np.float32)
    weight = np.asarray(weight, dtype=np.float32)
    biases = np.asarray(biases, dtype=np.float32)
    O_ = int(output_size)
    assert O_ == O and inputs.shape == (B, N * 64) and supports.shape == (2, N, N)

    nc = _build()

    # host staging: A^T, x0^T (bf16), prepped W (What/W1/2W2/W3/2W4), bias row
    at_np = np.ascontiguousarray(supports.transpose(0, 2, 1)).astype(NPBF16)
    x0 = np.concatenate(
        [inputs.reshape(B, N, 64), state.reshape(B, N, 64)], axis=2
    )  # [B, N, F]
    x0t = x0.transpose(0, 2, 1)  # [B, F, N] view
    w5 = weight.reshape(F, 5, O)
    wcn = np.empty((F, 5, O), np.float32)
    wcn[:, 0] = w5[:, 0] - w5[:, 2] - w5[:, 4]
    wcn[:, 1] = w5[:, 1]
    wcn[:, 2] = 2.0 * w5[:, 2]
    wcn[:, 3] = w5[:, 3]
    wcn[:, 4] = 2.0 * w5[:, 4]
    wcn = wcn.astype(NPBF16)

    in_maps = []
    for c in range(NCORES):
        in_maps.append(
            {
                "x0t": np.ascontiguousarray(x0t[c * BL : (c + 1) * BL]).astype(
                    NPBF16
                ),
                "at": at_np,
                "w": wcn,
            }
        )

    res = run_bass_kernel_spmd(
        nc, in_maps, core_ids=list(range(NCORES)), trace=_trace
    )
    kernel.last_result = res

    # out per core: [N, BL, O] -> full [B, N*O]
    parts = [res.results[c]["out"] for c in range(NCORES)]
    full = np.concatenate(parts, axis=1)  # [N, B, O]
    if np.any(biases):
        full = full + biases[None, None, :]
    return np.ascontiguousarray(full.transpose(1, 0, 2)).reshape(B, N * O_)


# revision 35
# speedup vs baseline: 1.1964x; 1.1964x over previous
"""DiffusionGraphConv Trainium2 kernel (bf16 v2).

Math (per batch b, support s, A = supports[s]):
  x0 = concat(inputs, state)                      # [N, F=128]
  reference out = sum_k x_k @ W_k  (+bias), k in {x0, x1_s0, x2_s0, x1_s1, x2_s1}
  with x1 = A x0, x2 = 2 A A x0 - x0, W_k = weight[f*5+k, :].

Restructured to avoid any on-chip transposes:
  out = x0 @ What + bias + sum_s A_s @ (x0 @ W1_s + A_s @ (x0 @ (2*W2_s)))
  with What = W_0 - W_2 - W_4 (host-prepped, as are the 2x scalings).

All matmul operands are bf16 (f32 PSUM accumulation): same 1 cycle/row
streaming as fp32r but half the DMA bytes (input head shrinks ~2x) and
FWL-accelerated LDWEIGHTS that hides fully under the streams.  Measured
end-to-end error vs the f32 reference is ~3.5e-3 on the max|err|/max|ref|
metric (gate 2e-2).

Layouts (per core, batch-sharded BL=8):
  x0tN  DRAM [b=8, F=128, m=1024] bf16  (host-staged transpose; lhsT tiles)
  atN   DRAM [s=2, m=1024, n=1024] bf16 (host-staged A^T; lhsT tiles)
  wcN   DRAM [F, 5, O] bf16 = [What, W1, 2*W2, W3, 2*W4]
  Sa step (b, mi): one matmul x0T-tile @ wc -> psum [128, 640]:
    [:,0:128] -> fins[mi] (f32 init), [:,128:640] -> st[:, mi, b, :] (bf16)
    st cols: [w1p_s0 | u_s0 | w1p_s1 | u_s1]
  v bank (s, ni, h):  v_s[ni, h] = A_s @ u_s + w1p_s   (psum f32 -> bf16)
  fin bank (s, ni, h): fin[ni, h] += A_s @ v_s; final s: DMA out [n, b, o]
"""

import sys as _sys
import types as _types

try:
    import antenv.axon_hooks  # noqa: F401
except Exception:
    try:
        import antenv as _antenv

        _m = _types.ModuleType("antenv.axon_hooks")
        _m._hook = None
        _m.set_axon_ntff_profile_hook = lambda h: setattr(_m, "_hook", h)
        _m.get_axon_ntff_profile_hook = lambda: _m._hook
        _sys.modules["antenv.axon_hooks"] = _m
        _antenv.axon_hooks = _m
    except Exception:
        pass

import ml_dtypes
import numpy as np

import concourse.mybir as mybir
import concourse.tile as tile
from concourse import bacc
from concourse.bass_utils import run_bass_kernel_spmd

NCORES = 8
B = 64
BL = B // NCORES  # 8 batches per core
N = 1024
F = 128
O = 128
NCH = N // 128  # 8 node chunks

BF16 = mybir.dt.bfloat16
F32 = mybir.dt.float32
NPBF16 = ml_dtypes.bfloat16

_CACHE = {}


def _build():
    if "nc" in _CACHE:
        return _CACHE["nc"]

    nc = bacc.Bacc(trn_type="TRN2", num_devices=NCORES, debug=False)

    x0t_d = nc.dram_tensor("x0t", [BL, F, N], BF16, kind="ExternalInput")
    at_d = nc.dram_tensor("at", [2, N, N], BF16, kind="ExternalInput")
    w_d = nc.dram_tensor("w", [F, 5, O], BF16, kind="ExternalInput")
    out_d = nc.dram_tensor("out", [N, BL, O], F32, kind="ExternalOutput")

    with tile.TileContext(nc) as tc:
        with (
            tc.tile_pool(name="big", bufs=1) as big,
            tc.tile_pool(name="small", bufs=1) as small,
            tc.tile_pool(name="ps_pool", bufs=1, space="PSUM") as ps_pool,
        ):
            # ---- persistent tiles ----
            wc = small.tile([F, 5, O], BF16)
            # x0t split into 4 tiles so early batches unblock Sa while the
            # rest still streams in (one DMA trigger per tile; triggers
            # serialize at ~0.6us each on an engine queue)
            x0t_g = [
                big.tile([F, 512], BF16, name="x0t_b0a"),
                big.tile([F, 512], BF16, name="x0t_b0b"),
                big.tile([F, 512], BF16, name="x0t_b1a"),
                big.tile([F, 512], BF16, name="x0t_b1b"),
                big.tile([F, 1, N], BF16, name="x0t_b2"),
                big.tile([F, 1, N], BF16, name="x0t_b3"),
                big.tile([F, 4, N], BF16, name="x0t_b47"),
            ]

            def x0chunk(b, mi):
                """[F, 128] lhsT tile for batch b, node chunk mi."""
                if b < 2:
                    g = 2 * b + (mi // 4)
                    return x0t_g[g][:, (mi % 4) * 128 : (mi % 4 + 1) * 128]
                if b < 4:
                    return x0t_g[2 + b][:, 0, mi * 128 : (mi + 1) * 128]
                return x0t_g[6][:, b - 4, mi * 128 : (mi + 1) * 128]

            at_t0 = big.tile([128, NCH, N], BF16, name="at_t0")
            at_t1 = big.tile([128, NCH, N], BF16, name="at_t1")
            # staging: [w1p_s0 | u_s0 | w1p_s1 | u_s1] per (mi, b)
            st = big.tile([128, NCH, BL, 512], BF16, name="st")
            v0 = big.tile([128, NCH, N], BF16, name="v0")
            v1 = big.tile([128, NCH, N], BF16, name="v1")
            fins = [big.tile([128, N], F32, name=f"fin{ni}") for ni in range(NCH)]

            # ---- PE warm-up: dummy matmuls during the DMA head so HAM
            # un-throttles (1.2 -> 2.4 GHz) before real work starts
            dummy = small.tile([128, 256], BF16)
            dsink = small.tile([128, 1], F32)
            nc.vector.memset(dummy[:], 0.0)
            for _ in range(10):
                pw = ps_pool.tile([128, 256], F32, name="ps_w", tag="ps", bufs=8)
                nc.tensor.matmul(
                    pw[:], dummy[:, 0:128], dummy[:], start=True, stop=True
                )
            nc.vector.tensor_copy(dsink[:], pw[:, 0:1])

            # ---- input DMAs: 7 triggers, all on ONE queue so the data
            # streams in strict priority order at full aggregate bandwidth
            # (a second active queue steals DMA-engine bandwidth from the
            # Sa-critical x0t transfers).
            nc.sync.dma_start(wc[:], w_d[:])
            x0r = x0t_d.rearrange("b f n -> f b n")
            for g in range(4):
                b, hh = g // 2, g % 2
                nc.sync.dma_start(
                    x0t_g[g][:], x0r[:, b, hh * 512 : (hh + 1) * 512]
                )
            nc.sync.dma_start(x0t_g[4][:], x0r[:, 2:3, :])
            nc.sync.dma_start(x0t_g[5][:], x0r[:, 3:4, :])
            nc.sync.dma_start(x0t_g[6][:], x0r[:, 4:8, :])
            nc.sync.dma_start(
                at_t0[:], at_d[0].rearrange("(mi p) n -> p mi n", p=128)
            )
            nc.sync.dma_start(
                at_t1[:], at_d[1].rearrange("(mi p) n -> p mi n", p=128)
            )

            # ---- Sa step (b, mi): x0T tile stationary, wc moving; two
            # 256-free matmuls fill one single-bank psum with the four
            # staged products [W1 | 2W2 | W3 | 2W4], then ONE contiguous
            # 512-wide f32->bf16 cast lands them in st.  (The x0@What term
            # is folded into the fin0 accumulation groups instead.)
            def sa_step(b, mi):
                cnt = b * NCH + mi
                ps = ps_pool.tile([128, 512], F32, name="ps_sa", tag="ps", bufs=8)
                x0_tile = x0chunk(b, mi)
                nc.tensor.matmul(
                    ps[:, 0:256], x0_tile, wc[:, 1:3, :], start=True, stop=True
                )
                nc.tensor.matmul(
                    ps[:, 256:512], x0_tile, wc[:, 3:5, :], start=True, stop=True
                )
                if cnt % 2 == 0:
                    nc.scalar.copy(st[:, mi, b, :], ps[:])
                else:
                    nc.vector.tensor_copy(st[:, mi, b, :], ps[:])

            # ---- v bank (s, ni, h): v_s[ni, h] = A_s @ u_s + w1p_s
            def v_bank(s, at_t, v, ni, h):
                pv = ps_pool.tile([128, 512], F32, name="ps_v", tag="ps", bufs=8)
                for mi in range(NCH):
                    nc.tensor.matmul(
                        pv[:],
                        at_t[:, mi, ni * 128 : (ni + 1) * 128],
                        st[:, mi, 4 * h : 4 * h + 4, 256 * s + 128 : 256 * s + 256],
                        start=(mi == 0),
                        stop=(mi == NCH - 1),
                    )
                nc.vector.tensor_add(
                    v[:, ni, h * 512 : (h + 1) * 512],
                    pv[:],
                    st[:, ni, 4 * h : 4 * h + 4, 256 * s : 256 * s + 128],
                )

            # ---- fin bank (s, ni, h): fin[ni, h] += A_s @ v_s.
            # s=0 additionally accumulates the x0@What term (one 128-free
            # matmul per batch in the group) and writes fins fresh from the
            # psum; s=1 does fins += psum and DMAs the result out.  The very
            # last bank splits its add+DMA into 4 batch slices so the
            # exposed tail after the final matmul stays short.
            def fin_bank(s, at_t, v, ni, h):
                pf = ps_pool.tile([128, 512], F32, name="ps_f", tag="ps", bufs=8)
                for mi in range(NCH):
                    nc.tensor.matmul(
                        pf[:],
                        at_t[:, mi, ni * 128 : (ni + 1) * 128],
                        v[:, mi, h * 512 : (h + 1) * 512],
                        start=(mi == 0),
                        stop=(s == 1 and mi == NCH - 1),
                    )
                if s == 0:
                    for k in range(4):
                        b = 4 * h + k
                        nc.tensor.matmul(
                            pf[:, k * 128 : (k + 1) * 128],
                            x0chunk(b, ni),
                            wc[:, 0, :],
                            start=False,
                            stop=(k == 3),
                        )
                fslc = fins[ni][:, h * 512 : (h + 1) * 512]
                last = s == 1 and ni == NCH - 1
                if s == 0:
                    nc.vector.tensor_copy(fslc, pf[:])
                elif not last:
                    nc.vector.tensor_add(fslc, fslc, pf[:])
                    nc.sync.dma_start(
                        out_d[ni * 128 : (ni + 1) * 128, 4 * h : 4 * h + 4, :],
                        fslc,
                    )
                else:
                    # tail: one add, then the DMA split by node quarter
                    # (keeps 2KB-contiguous rows) fanned over idle engine
                    # queues so the triggers fire in parallel
                    nc.vector.tensor_add(fslc, fslc, pf[:])
                    trig = [nc.scalar, nc.gpsimd, nc.sync, nc.scalar]
                    for q in range(4):
                        p0 = q * 32
                        trig[q].dma_start(
                            out_d[
                                ni * 128 + p0 : ni * 128 + p0 + 32,
                                4 * h : 4 * h + 4,
                                :,
                            ],
                            fins[ni][p0 : p0 + 32, h * 512 : (h + 1) * 512],
                        )

            # ---- schedule ----
            # Sa b0-6 alone (runs during the x0t/at0 DMA), then v0 h=0
            # banks interleaved with Sa b7 (1 step per bank); delaying
            # v0 start this way lets the at0 DMA finish first.
            for b in range(7):
                for mi in range(NCH):
                    sa_step(b, mi)
            for ni in range(NCH):
                v_bank(0, at_t0, v0, ni, 0)
                sa_step(7, ni)
            for ni in range(NCH):
                v_bank(0, at_t0, v0, ni, 1)

            for ni in range(NCH):
                for h in range(2):
                    fin_bank(0, at_t0, v0, ni, h)

            for h in range(2):
                for ni in range(NCH):
                    v_bank(1, at_t1, v1, ni, h)
            for ni in range(NCH):
                for h in range(2):
                    fin_bank(1, at_t1, v1, ni, h)

    nc.compile()
    _CACHE["nc"] = nc
    return nc


def kernel(supports, inputs, state, weight, biases, output_size, _trace=False):
    supports = np.asarray(supports, dtype=np.float32)
    inputs = np.asarray(inputs, dtype=np.float32)
    state = np.asarray(state, dtype=np.float32)
    weight = np.asarray(weight, dtype=np.float32)
    biases = np.asarray(biases, dtype=np.float32)
    O_ = int(output_size)
    assert O_ == O and inputs.shape == (B, N * 64) and supports.shape == (2, N, N)

    nc = _build()

    # host staging: A^T, x0^T (bf16), prepped W (What/W1/2W2/W3/2W4), bias row
    at_np = np.ascontiguousarray(supports.transpose(0, 2, 1)).astype(NPBF16)
    x0 = np.concatenate(
        [inputs.reshape(B, N, 64), state.reshape(B, N, 64)], axis=2
    )  # [B, N, F]
    x0t = x0.transpose(0, 2, 1)  # [B, F, N] view
    w5 = weight.reshape(F, 5, O)
    wcn = np.empty((F, 5, O), np.float32)
    wcn[:, 0] = w5[:, 0] - w5[:, 2] - w5[:, 4]
    wcn[:, 1] = w5[:, 1]
    wcn[:, 2] = 2.0 * w5[:, 2]
    wcn[:, 3] = w5[:, 3]
    wcn[:, 4] = 2.0 * w5[:, 4]
    wcn = wcn.astype(NPBF16)

    in_maps = []
    for c in range(NCORES):
        in_maps.append(
            {
                "x0t": np.ascontiguousarray(x0t[c * BL : (c + 1) * BL]).astype(
                    NPBF16
                ),
                "at": at_np,
                "w": wcn,
            }
        )

    res = run_bass_kernel_spmd(
        nc, in_maps, core_ids=list(range(NCORES)), trace=_trace
    )
    kernel.last_result = res

    # out per core: [N, BL, O] -> full [B, N*O]
    parts = [res.results[c]["out"] for c in range(NCORES)]
    full = np.concatenate(parts, axis=1)  # [N, B, O]
    if np.any(biases):
        full = full + biases[None, None, :]
    return np.ascontiguousarray(full.transpose(1, 0, 2)).reshape(B, N * O_)


# revision 36
# speedup vs baseline: 1.1984x; 1.0017x over previous
"""DiffusionGraphConv Trainium2 kernel (bf16 v2).

Math (per batch b, support s, A = supports[s]):
  x0 = concat(inputs, state)                      # [N, F=128]
  reference out = sum_k x_k @ W_k  (+bias), k in {x0, x1_s0, x2_s0, x1_s1, x2_s1}
  with x1 = A x0, x2 = 2 A A x0 - x0, W_k = weight[f*5+k, :].

Restructured to avoid any on-chip transposes:
  out = x0 @ What + bias + sum_s A_s @ (x0 @ W1_s + A_s @ (x0 @ (2*W2_s)))
  with What = W_0 - W_2 - W_4 (host-prepped, as are the 2x scalings).

All matmul operands are bf16 (f32 PSUM accumulation): same 1 cycle/row
streaming as fp32r but half the DMA bytes (input head shrinks ~2x) and
FWL-accelerated LDWEIGHTS that hides fully under the streams.  Measured
end-to-end error vs the f32 reference is ~3.5e-3 on the max|err|/max|ref|
metric (gate 2e-2).

Layouts (per core, batch-sharded BL=8):
  x0tN  DRAM [b=8, F=128, m=1024] bf16  (host-staged transpose; lhsT tiles)
  atN   DRAM [s=2, m=1024, n=1024] bf16 (host-staged A^T; lhsT tiles)
  wcN   DRAM [F, 5, O] bf16 = [What, W1, 2*W2, W3, 2*W4]
  Sa step (b, mi): one matmul x0T-tile @ wc -> psum [128, 640]:
    [:,0:128] -> fins[mi] (f32 init), [:,128:640] -> st[:, mi, b, :] (bf16)
    st cols: [w1p_s0 | u_s0 | w1p_s1 | u_s1]
  v bank (s, ni, h):  v_s[ni, h] = A_s @ u_s + w1p_s   (psum f32 -> bf16)
  fin bank (s, ni, h): fin[ni, h] += A_s @ v_s; final s: DMA out [n, b, o]
"""

import sys as _sys
import types as _types

try:
    import antenv.axon_hooks  # noqa: F401
except Exception:
    try:
        import antenv as _antenv

        _m = _types.ModuleType("antenv.axon_hooks")
        _m._hook = None
        _m.set_axon_ntff_profile_hook = lambda h: setattr(_m, "_hook", h)
        _m.get_axon_ntff_profile_hook = lambda: _m._hook
        _sys.modules["antenv.axon_hooks"] = _m
        _antenv.axon_hooks = _m
    except Exception:
        pass

import ml_dtypes
import numpy as np

import concourse.mybir as mybir
import concourse.tile as tile
from concourse import bacc
from concourse.bass_utils import run_bass_kernel_spmd

NCORES = 8
B = 64
BL = B // NCORES  # 8 batches per core
N = 1024
F = 128
O = 128
NCH = N // 128  # 8 node chunks

BF16 = mybir.dt.bfloat16
F32 = mybir.dt.float32
NPBF16 = ml_dtypes.bfloat16

_CACHE = {}


def _build():
    if "nc" in _CACHE:
        return _CACHE["nc"]

    nc = bacc.Bacc(trn_type="TRN2", num_devices=NCORES, debug=False)

    x0t_d = nc.dram_tensor("x0t", [BL, F, N], BF16, kind="ExternalInput")
    at_d = nc.dram_tensor("at", [2, N, N], BF16, kind="ExternalInput")
    w_d = nc.dram_tensor("w", [F, 5, O], BF16, kind="ExternalInput")
    out_d = nc.dram_tensor("out", [N, BL, O], F32, kind="ExternalOutput")

    with tile.TileContext(nc) as tc:
        with (
            tc.tile_pool(name="big", bufs=1) as big,
            tc.tile_pool(name="small", bufs=1) as small,
            tc.tile_pool(name="ps_pool", bufs=1, space="PSUM") as ps_pool,
        ):
            # ---- persistent tiles ----
            wc = small.tile([F, 5, O], BF16)
            # x0t split into 4 tiles so early batches unblock Sa while the
            # rest still streams in (one DMA trigger per tile; triggers
            # serialize at ~0.6us each on an engine queue)
            x0t_g = [
                big.tile([F, 512], BF16, name="x0t_b0a"),
                big.tile([F, 512], BF16, name="x0t_b0b"),
                big.tile([F, 512], BF16, name="x0t_b1a"),
                big.tile([F, 512], BF16, name="x0t_b1b"),
                big.tile([F, 1, N], BF16, name="x0t_b2"),
                big.tile([F, 1, N], BF16, name="x0t_b3"),
                big.tile([F, 4, N], BF16, name="x0t_b47"),
            ]

            def x0chunk(b, mi):
                """[F, 128] lhsT tile for batch b, node chunk mi."""
                if b < 2:
                    g = 2 * b + (mi // 4)
                    return x0t_g[g][:, (mi % 4) * 128 : (mi % 4 + 1) * 128]
                if b < 4:
                    return x0t_g[2 + b][:, 0, mi * 128 : (mi + 1) * 128]
                return x0t_g[6][:, b - 4, mi * 128 : (mi + 1) * 128]

            at_t0 = big.tile([128, NCH, N], BF16, name="at_t0")
            at_t1 = big.tile([128, NCH, N], BF16, name="at_t1")
            # staging: [w1p_s0 | u_s0 | w1p_s1 | u_s1] per (mi, b)
            st = big.tile([128, NCH, BL, 512], BF16, name="st")
            v0 = big.tile([128, NCH, N], BF16, name="v0")
            v1 = big.tile([128, NCH, N], BF16, name="v1")
            fins = [big.tile([128, N], F32, name=f"fin{ni}") for ni in range(NCH)]

            # ---- PE warm-up: dummy matmuls during the DMA head so HAM
            # un-throttles (1.2 -> 2.4 GHz) before real work starts
            dummy = small.tile([128, 256], BF16)
            dsink = small.tile([128, 1], F32)
            nc.vector.memset(dummy[:], 0.0)
            for _ in range(10):
                pw = ps_pool.tile([128, 256], F32, name="ps_w", tag="ps", bufs=8)
                nc.tensor.matmul(
                    pw[:], dummy[:, 0:128], dummy[:], start=True, stop=True
                )
            nc.vector.tensor_copy(dsink[:], pw[:, 0:1])

            # ---- input DMAs: 7 triggers, all on ONE queue so the data
            # streams in strict priority order at full aggregate bandwidth
            # (a second active queue steals DMA-engine bandwidth from the
            # Sa-critical x0t transfers).
            nc.sync.dma_start(wc[:], w_d[:])
            x0r = x0t_d.rearrange("b f n -> f b n")
            for g in range(4):
                b, hh = g // 2, g % 2
                nc.sync.dma_start(
                    x0t_g[g][:], x0r[:, b, hh * 512 : (hh + 1) * 512]
                )
            nc.sync.dma_start(x0t_g[4][:], x0r[:, 2:3, :])
            nc.sync.dma_start(x0t_g[5][:], x0r[:, 3:4, :])
            nc.sync.dma_start(x0t_g[6][:], x0r[:, 4:8, :])
            nc.sync.dma_start(
                at_t0[:], at_d[0].rearrange("(mi p) n -> p mi n", p=128)
            )
            nc.sync.dma_start(
                at_t1[:], at_d[1].rearrange("(mi p) n -> p mi n", p=128)
            )

            # ---- Sa step (b, mi): x0T tile stationary, wc moving; two
            # 256-free matmuls fill one single-bank psum with the four
            # staged products [W1 | 2W2 | W3 | 2W4], then ONE contiguous
            # 512-wide f32->bf16 cast lands them in st.  (The x0@What term
            # is folded into the fin0 accumulation groups instead.)
            def sa_step(b, mi):
                cnt = b * NCH + mi
                ps = ps_pool.tile([128, 512], F32, name="ps_sa", tag="ps", bufs=8)
                nc.tensor.matmul(
                    ps[:], x0chunk(b, mi), wc[:, 1:5, :], start=True, stop=True
                )
                if cnt % 2 == 0:
                    nc.scalar.copy(st[:, mi, b, :], ps[:])
                else:
                    nc.vector.tensor_copy(st[:, mi, b, :], ps[:])

            # ---- v bank (s, ni, h): v_s[ni, h] = A_s @ u_s + w1p_s
            def v_bank(s, at_t, v, ni, h):
                pv = ps_pool.tile([128, 512], F32, name="ps_v", tag="ps", bufs=8)
                for mi in range(NCH):
                    nc.tensor.matmul(
                        pv[:],
                        at_t[:, mi, ni * 128 : (ni + 1) * 128],
                        st[:, mi, 4 * h : 4 * h + 4, 256 * s + 128 : 256 * s + 256],
                        start=(mi == 0),
                        stop=(mi == NCH - 1),
                    )
                nc.vector.tensor_add(
                    v[:, ni, h * 512 : (h + 1) * 512],
                    pv[:],
                    st[:, ni, 4 * h : 4 * h + 4, 256 * s : 256 * s + 128],
                )

            # ---- fin bank (s, ni, h): fin[ni, h] += A_s @ v_s.
            # s=0 additionally accumulates the x0@What term (one 128-free
            # matmul per batch in the group) and writes fins fresh from the
            # psum; s=1 does fins += psum and DMAs the result out.  The very
            # last bank splits its add+DMA into 4 batch slices so the
            # exposed tail after the final matmul stays short.
            def fin_bank(s, at_t, v, ni, h):
                pf = ps_pool.tile([128, 512], F32, name="ps_f", tag="ps", bufs=8)
                for mi in range(NCH):
                    nc.tensor.matmul(
                        pf[:],
                        at_t[:, mi, ni * 128 : (ni + 1) * 128],
                        v[:, mi, h * 512 : (h + 1) * 512],
                        start=(mi == 0),
                        stop=(s == 1 and mi == NCH - 1),
                    )
                if s == 0:
                    for k in range(4):
                        b = 4 * h + k
                        nc.tensor.matmul(
                            pf[:, k * 128 : (k + 1) * 128],
                            x0chunk(b, ni),
                            wc[:, 0, :],
                            start=False,
                            stop=(k == 3),
                        )
                fslc = fins[ni][:, h * 512 : (h + 1) * 512]
                last = s == 1 and ni == NCH - 1
                if s == 0:
                    nc.vector.tensor_copy(fslc, pf[:])
                elif not last:
                    nc.vector.tensor_add(fslc, fslc, pf[:])
                    nc.sync.dma_start(
                        out_d[ni * 128 : (ni + 1) * 128, 4 * h : 4 * h + 4, :],
                        fslc,
                    )
                else:
                    # tail: one add, then the DMA split by node quarter
                    # (keeps 2KB-contiguous rows) fanned over idle engine
                    # queues so the triggers fire in parallel
                    nc.vector.tensor_add(fslc, fslc, pf[:])
                    trig = [nc.scalar, nc.gpsimd, nc.sync, nc.scalar]
                    for q in range(4):
                        p0 = q * 32
                        trig[q].dma_start(
                            out_d[
                                ni * 128 + p0 : ni * 128 + p0 + 32,
                                4 * h : 4 * h + 4,
                                :,
                            ],
                            fins[ni][p0 : p0 + 32, h * 512 : (h + 1) * 512],
                        )

            # ---- schedule ----
            # Sa b0-6 alone (runs during the x0t/at0 DMA), then v0 h=0
            # banks interleaved with Sa b7 (1 step per bank); delaying
            # v0 start this way lets the at0 DMA finish first.
            for b in range(7):
                for mi in range(NCH):
                    sa_step(b, mi)
            for ni in range(NCH):
                v_bank(0, at_t0, v0, ni, 0)
                sa_step(7, ni)
            for ni in range(NCH):
                v_bank(0, at_t0, v0, ni, 1)

            for ni in range(NCH):
                for h in range(2):
                    fin_bank(0, at_t0, v0, ni, h)

            for h in range(2):
                for ni in range(NCH):
                    v_bank(1, at_t1, v1, ni, h)
            for ni in range(NCH):
                for h in range(2):
                    fin_bank(1, at_t1, v1, ni, h)

    nc.compile()
    _CACHE["nc"] = nc
    return nc


def kernel(supports, inputs, state, weight, biases, output_size, _trace=False):
    supports = np.asarray(supports, dtype=np.float32)
    inputs = np.asarray(inputs, dtype=np.float32)
    state = np.asarray(state, dtype=np.float32)
    weight = np.asarray(weight, dtype=np.float32)
    biases = np.asarray(biases, dtype=np.float32)
    O_ = int(output_size)
    assert O_ == O and inputs.shape == (B, N * 64) and supports.shape == (2, N, N)

    nc = _build()

    # host staging: A^T, x0^T (bf16), prepped W (What/W1/2W2/W3/2W4), bias row
    at_np = np.ascontiguousarray(supports.transpose(0, 2, 1)).astype(NPBF16)
    x0 = np.concatenate(
        [inputs.reshape(B, N, 64), state.reshape(B, N, 64)], axis=2
    )  # [B, N, F]
    x0t = x0.transpose(0, 2, 1)  # [B, F, N] view
    w5 = weight.reshape(F, 5, O)
    wcn = np.empty((F, 5, O), np.float32)
    wcn[:, 0] = w5[:, 0] - w5[:, 2] - w5[:, 4]
    wcn[:, 1] = w5[:, 1]
    wcn[:, 2] = 2.0 * w5[:, 2]
    wcn[:, 3] = w5[:, 3]
    wcn[:, 4] = 2.0 * w5[:, 4]
    wcn = wcn.astype(NPBF16)

    in_maps = []
    for c in range(NCORES):
        in_maps.append(
            {
                "x0t": np.ascontiguousarray(x0t[c * BL : (c + 1) * BL]).astype(
                    NPBF16
                ),
                "at": at_np,
                "w": wcn,
            }
        )

    res = run_bass_kernel_spmd(
        nc, in_maps, core_ids=list(range(NCORES)), trace=_trace
    )
    kernel.last_result = res

    # out per core: [N, BL, O] -> full [B, N*O]
    parts = [res.results[c]["out"] for c in range(NCORES)]
    full = np.concatenate(parts, axis=1)  # [N, B, O]
    if np.any(biases):
        full = full + biases[None, None, :]
    return np.ascontiguousarray(full.transpose(1, 0, 2)).reshape(B, N * O_)


# revision 37
# speedup vs baseline: 1.2026x; 1.0035x over previous
"""DiffusionGraphConv Trainium2 kernel (bf16).

Math (per batch b, support s, A = supports[s]):
  x0 = concat(inputs, state)                      # [N, F=128]
  reference out = sum_k x_k @ W_k  (+bias), k in {x0, x1_s0, x2_s0, x1_s1, x2_s1}
  with x1 = A x0, x2 = 2 A A x0 - x0, W_k = weight[f*5+k, :].

Restructured to avoid any on-chip transposes:
  out = x0 @ What + bias + sum_s A_s @ (x0 @ W1_s + A_s @ (x0 @ (2*W2_s)))
  with What = W_0 - W_2 - W_4 (host-prepped, as are the 2x scalings;
  bias, zero in this problem, is applied on the host when nonzero).

All matmul operands are bf16 (f32 PSUM accumulation): same 1 cycle/row
streaming as fp32r but half the DMA bytes (halves the input head, which
is trigger- and ramp-limited) and FWL-accelerated LDWEIGHTS that hides
fully under the 512-free streams.  Measured end-to-end error vs the f32
reference is 4.5e-3 on the max|err|/max|ref| metric (gate 2e-2).

Layouts (per core, batch-sharded BL=8):
  x0t  DRAM [b=8, F=128, m=1024] bf16  (host-staged transpose; lhsT tiles,
       loaded as 7 chunks sized so Sa's critical path lands first)
  at   DRAM [s=2, m=1024, n=1024] bf16 (host-staged A^T; lhsT tiles)
  w    DRAM [F, 5, O] bf16 = [What, W1, 2*W2, W3, 2*W4]
  Sa step (b, mi): one 512-free matmul x0T-tile @ wc[:,1:5] -> psum,
    cast to st[:, mi, b, :] = [w1p_s0 | u_s0 | w1p_s1 | u_s1] (bf16)
  v bank (s, ni, h):  v_s[ni, h] = A_s @ u_s + w1p_s   (psum f32 -> bf16)
  fin bank (s, ni, h): s=0: fins[ni,h] = A_0 @ v_0 + x0 @ What (What rides
    the same accumulation group, one 128-free matmul per batch);
    s=1: fins += A_1 @ v_1, then DMA out [n, b, o] (node-quartered on the
    final bank so the exposed tail stays short).

Schedule: 10 warmup matmuls (HAM un-throttle) -> Sa b0-b6 during the
input DMA -> v0 h0 interleaved with Sa b7 -> v0 h1 -> fin0 -> v1 -> fin1.
"""

import sys as _sys
import types as _types

try:
    import antenv.axon_hooks  # noqa: F401
except Exception:
    try:
        import antenv as _antenv

        _m = _types.ModuleType("antenv.axon_hooks")
        _m._hook = None
        _m.set_axon_ntff_profile_hook = lambda h: setattr(_m, "_hook", h)
        _m.get_axon_ntff_profile_hook = lambda: _m._hook
        _sys.modules["antenv.axon_hooks"] = _m
        _antenv.axon_hooks = _m
    except Exception:
        pass

import ml_dtypes
import numpy as np

import concourse.mybir as mybir
import concourse.tile as tile
from concourse import bacc
from concourse.bass_utils import run_bass_kernel_spmd

NCORES = 8
B = 64
BL = B // NCORES  # 8 batches per core
N = 1024
F = 128
O = 128
NCH = N // 128  # 8 node chunks

BF16 = mybir.dt.bfloat16
F32 = mybir.dt.float32
NPBF16 = ml_dtypes.bfloat16

_CACHE = {}


def _build():
    if "nc" in _CACHE:
        return _CACHE["nc"]

    nc = bacc.Bacc(trn_type="TRN2", num_devices=NCORES, debug=False)

    x0t_d = nc.dram_tensor("x0t", [BL, F, N], BF16, kind="ExternalInput")
    at_d = nc.dram_tensor("at", [2, N, N], BF16, kind="ExternalInput")
    w_d = nc.dram_tensor("w", [F, 5, O], BF16, kind="ExternalInput")
    out_d = nc.dram_tensor("out", [N, BL, O], F32, kind="ExternalOutput")

    with tile.TileContext(nc) as tc:
        with (
            tc.tile_pool(name="big", bufs=1) as big,
            tc.tile_pool(name="small", bufs=1) as small,
            tc.tile_pool(name="ps_pool", bufs=1, space="PSUM") as ps_pool,
        ):
            # ---- persistent tiles ----
            wc = small.tile([F, 5, O], BF16)
            # x0t split into 4 tiles so early batches unblock Sa while the
            # rest still streams in (one DMA trigger per tile; triggers
            # serialize at ~0.6us each on an engine queue)
            x0t_g = [
                big.tile([F, 512], BF16, name="x0t_b0a"),
                big.tile([F, 512], BF16, name="x0t_b0b"),
                big.tile([F, 512], BF16, name="x0t_b1a"),
                big.tile([F, 512], BF16, name="x0t_b1b"),
                big.tile([F, 1, N], BF16, name="x0t_b2"),
                big.tile([F, 1, N], BF16, name="x0t_b3"),
                big.tile([F, 4, N], BF16, name="x0t_b47"),
            ]

            def x0chunk(b, mi):
                """[F, 128] lhsT tile for batch b, node chunk mi."""
                if b < 2:
                    g = 2 * b + (mi // 4)
                    return x0t_g[g][:, (mi % 4) * 128 : (mi % 4 + 1) * 128]
                if b < 4:
                    return x0t_g[2 + b][:, 0, mi * 128 : (mi + 1) * 128]
                return x0t_g[6][:, b - 4, mi * 128 : (mi + 1) * 128]

            at_t0 = big.tile([128, NCH, N], BF16, name="at_t0")
            at_t1 = big.tile([128, NCH, N], BF16, name="at_t1")
            # staging: [w1p_s0 | u_s0 | w1p_s1 | u_s1] per (mi, b)
            st = big.tile([128, NCH, BL, 512], BF16, name="st")
            v0 = big.tile([128, NCH, N], BF16, name="v0")
            v1 = big.tile([128, NCH, N], BF16, name="v1")
            fins = [big.tile([128, N], F32, name=f"fin{ni}") for ni in range(NCH)]

            # ---- PE warm-up: dummy matmuls during the DMA head so HAM
            # un-throttles (1.2 -> 2.4 GHz) before real work starts
            dummy = small.tile([128, 256], BF16)
            dsink = small.tile([128, 1], F32)
            nc.vector.memset(dummy[:], 0.0)
            for _ in range(10):
                pw = ps_pool.tile([128, 256], F32, name="ps_w", tag="ps", bufs=8)
                nc.tensor.matmul(
                    pw[:], dummy[:, 0:128], dummy[:], start=True, stop=True
                )
            nc.vector.tensor_copy(dsink[:], pw[:, 0:1])

            # ---- input DMAs: 7 triggers, all on ONE queue so the data
            # streams in strict priority order at full aggregate bandwidth
            # (a second active queue steals DMA-engine bandwidth from the
            # Sa-critical x0t transfers).
            nc.sync.dma_start(wc[:], w_d[:])
            x0r = x0t_d.rearrange("b f n -> f b n")
            for g in range(4):
                b, hh = g // 2, g % 2
                nc.sync.dma_start(
                    x0t_g[g][:], x0r[:, b, hh * 512 : (hh + 1) * 512]
                )
            nc.sync.dma_start(x0t_g[4][:], x0r[:, 2:3, :])
            nc.sync.dma_start(x0t_g[5][:], x0r[:, 3:4, :])
            nc.sync.dma_start(x0t_g[6][:], x0r[:, 4:8, :])
            nc.sync.dma_start(
                at_t0[:], at_d[0].rearrange("(mi p) n -> p mi n", p=128)
            )
            nc.sync.dma_start(
                at_t1[:], at_d[1].rearrange("(mi p) n -> p mi n", p=128)
            )

            # ---- Sa step (b, mi): x0T tile stationary, wc moving; two
            # 256-free matmuls fill one single-bank psum with the four
            # staged products [W1 | 2W2 | W3 | 2W4], then ONE contiguous
            # 512-wide f32->bf16 cast lands them in st.  (The x0@What term
            # is folded into the fin0 accumulation groups instead.)
            def sa_step(b, mi):
                cnt = b * NCH + mi
                ps = ps_pool.tile([128, 512], F32, name="ps_sa", tag="ps", bufs=8)
                nc.tensor.matmul(
                    ps[:], x0chunk(b, mi), wc[:, 1:5, :], start=True, stop=True
                )
                if cnt % 2 == 0:
                    nc.scalar.copy(st[:, mi, b, :], ps[:])
                else:
                    nc.vector.tensor_copy(st[:, mi, b, :], ps[:])

            # ---- v bank (s, ni, h): v_s[ni, h] = A_s @ u_s + w1p_s
            def v_bank(s, at_t, v, ni, h):
                pv = ps_pool.tile([128, 512], F32, name="ps_v", tag="ps", bufs=8)
                for mi in range(NCH):
                    nc.tensor.matmul(
                        pv[:],
                        at_t[:, mi, ni * 128 : (ni + 1) * 128],
                        st[:, mi, 4 * h : 4 * h + 4, 256 * s + 128 : 256 * s + 256],
                        start=(mi == 0),
                        stop=(mi == NCH - 1),
                    )
                nc.vector.tensor_add(
                    v[:, ni, h * 512 : (h + 1) * 512],
                    pv[:],
                    st[:, ni, 4 * h : 4 * h + 4, 256 * s : 256 * s + 128],
                )

            # ---- fin bank (s, ni, h): fin[ni, h] += A_s @ v_s.
            # s=0 additionally accumulates the x0@What term (one 128-free
            # matmul per batch in the group) and writes fins fresh from the
            # psum; s=1 does fins += psum and DMAs the result out.  The very
            # last bank splits its add+DMA into 4 batch slices so the
            # exposed tail after the final matmul stays short.
            def fin_bank(s, at_t, v, ni, h):
                pf = ps_pool.tile([128, 512], F32, name="ps_f", tag="ps", bufs=8)
                for mi in range(NCH):
                    nc.tensor.matmul(
                        pf[:],
                        at_t[:, mi, ni * 128 : (ni + 1) * 128],
                        v[:, mi, h * 512 : (h + 1) * 512],
                        start=(mi == 0),
                        stop=(s == 1 and mi == NCH - 1),
                    )
                if s == 0:
                    for k in range(4):
                        b = 4 * h + k
                        nc.tensor.matmul(
                            pf[:, k * 128 : (k + 1) * 128],
                            x0chunk(b, ni),
                            wc[:, 0, :],
                            start=False,
                            stop=(k == 3),
                        )
                fslc = fins[ni][:, h * 512 : (h + 1) * 512]
                last = s == 1 and ni == NCH - 1
                if s == 0:
                    nc.vector.tensor_copy(fslc, pf[:])
                elif not last:
                    nc.vector.tensor_add(fslc, fslc, pf[:])
                    nc.sync.dma_start(
                        out_d[ni * 128 : (ni + 1) * 128, 4 * h : 4 * h + 4, :],
                        fslc,
                    )
                else:
                    # tail: one add, then the DMA split by node quarter
                    # (keeps 2KB-contiguous rows) fanned over idle engine
                    # queues so the triggers fire in parallel
                    nc.vector.tensor_add(fslc, fslc, pf[:])
                    trig = [nc.scalar, nc.gpsimd, nc.sync, nc.scalar]
                    for q in range(4):
                        p0 = q * 32
                        trig[q].dma_start(
                            out_d[
                                ni * 128 + p0 : ni * 128 + p0 + 32,
                                4 * h : 4 * h + 4,
                                :,
                            ],
                            fins[ni][p0 : p0 + 32, h * 512 : (h + 1) * 512],
                        )

            # ---- schedule ----
            # Sa b0-6 alone (runs during the x0t/at0 DMA), then v0 h=0
            # banks interleaved with Sa b7 (1 step per bank); delaying
            # v0 start this way lets the at0 DMA finish first.
            for b in range(7):
                for mi in range(NCH):
                    sa_step(b, mi)
            for ni in range(NCH):
                v_bank(0, at_t0, v0, ni, 0)
                sa_step(7, ni)
            for ni in range(NCH):
                v_bank(0, at_t0, v0, ni, 1)

            for ni in range(NCH):
                for h in range(2):
                    fin_bank(0, at_t0, v0, ni, h)

            for h in range(2):
                for ni in range(NCH):
                    v_bank(1, at_t1, v1, ni, h)
            for ni in range(NCH):
                for h in range(2):
                    fin_bank(1, at_t1, v1, ni, h)

    nc.compile()
    _CACHE["nc"] = nc
    return nc


def kernel(supports, inputs, state, weight, biases, output_size, _trace=False):
    supports = np.asarray(supports, dtype=np.float32)
    inputs = np.asarray(inputs, dtype=np.float32)
    state = np.asarray(state, dtype=np.float32)
    weight = np.asarray(weight, dtype=np.float32)
    biases = np.asarray(biases, dtype=np.float32)
    O_ = int(output_size)
    assert O_ == O and inputs.shape == (B, N * 64) and supports.shape == (2, N, N)

    nc = _build()

    # host staging: A^T, x0^T (bf16), prepped W (What/W1/2W2/W3/2W4), bias row
    at_np = np.ascontiguousarray(supports.transpose(0, 2, 1)).astype(NPBF16)
    x0 = np.concatenate(
        [inputs.reshape(B, N, 64), state.reshape(B, N, 64)], axis=2
    )  # [B, N, F]
    x0t = x0.transpose(0, 2, 1)  # [B, F, N] view
    w5 = weight.reshape(F, 5, O)
    wcn = np.empty((F, 5, O), np.float32)
    wcn[:, 0] = w5[:, 0] - w5[:, 2] - w5[:, 4]
    wcn[:, 1] = w5[:, 1]
    wcn[:, 2] = 2.0 * w5[:, 2]
    wcn[:, 3] = w5[:, 3]
    wcn[:, 4] = 2.0 * w5[:, 4]
    wcn = wcn.astype(NPBF16)

    in_maps = []
    for c in range(NCORES):
        in_maps.append(
            {
                "x0t": np.ascontiguousarray(x0t[c * BL : (c + 1) * BL]).astype(
                    NPBF16
                ),
                "at": at_np,
                "w": wcn,
            }
        )

    res = run_bass_kernel_spmd(
        nc, in_maps, core_ids=list(range(NCORES)), trace=_trace
    )
    kernel.last_result = res

    # out per core: [N, BL, O] -> full [B, N*O]
    parts = [res.results[c]["out"] for c in range(NCORES)]
    full = np.concatenate(parts, axis=1)  # [N, B, O]
    if np.any(biases):
        full = full + biases[None, None, :]
    return np.ascontiguousarray(full.transpose(1, 0, 2)).reshape(B, N * O_)
